# revision 46
# baseline (speedup 1.0000x reference)
"""Trainium2 Bass kernel for transfer-matrix reflectometry (DifferentiableKLA).

Math: for each batch row b (7 film thicknesses) and wavelength w, the
reference computes R = |M10/M00|^2 of an 8-interface transfer-matrix
product. Dividing the backward (Rouard) recurrence through by v and
rescaling by conj(E_i) each step (the overall complex scale cancels in the
ratio) gives the division-free form over x = 1/lambda:

    u, v   <- r_8, 1
    for i = 7..1:   T = v * conj(E_i);  u' = r_i*T + u;  v' = T + r_i*u
    R = |u|^2 / |v|^2,   E_i = exp(i * theta_i), theta_i = 2*pi*x*g_i,
    g_i = 2*n_i*d_i,   r_i = (n_{i-1}-n_i)/(n_{i-1}+n_i)  (real scalars
    since the refractive indices are wavelength-constant and real).

Key accel trick: U2=|u|^2 and V2=|v|^2 are *exactly band-limited* in x
(finite exponential sums, |freq| <= 4*pi*sum_j g_j), so they are computed
on a coarse WC-point grid in x and upsampled to the 801-point grid with a
precomputed band-limited least-squares matrix — evaluated as a TensorE
matmul that simultaneously transposes to the [batch, wavelength] output
layout. Only the final reciprocal+multiply runs on the fine grid.

Layout: coarse grid on partitions, batch along the free dim, two
independent batch halves packed as 2 x WC=64 partition groups (rank-2
matmul builds theta for both halves at once).
"""

import math

import numpy as np

W = 801
B = 16384
NCORES = 8
BPC = B // NCORES  # 2048 batch rows per core
WC = 64  # coarse wavelength nodes
PACK = 2  # batch halves packed along partitions
NPART = WC * PACK  # 128
NBS = (512, 512)  # chunk widths (step-interleaved in the recurrence)
COFF = (0, 512)  # column offsets per chunk
NCHUNK = len(NBS)
EXT = 0.05  # coarse-grid extension fraction beyond the x band
REG = 1e-10  # LS ridge regularization
MFREQ = 6000  # frequency samples for the LS fit
TWO_PI = 2.0 * math.pi
# worst-case band limit for d < 200nm stacks (4*pi*sum(2*n_j*200)), padded
B_DESIGN = 2.0 * TWO_PI * (2.0 * (1.46 * 4 + 2.0 * 3) * 200.0) * 1.02

_interp_cache: dict = {}
_program_cache: dict = {}
_last_launch: dict = {}  # debug/profiling hook (nc, in_maps of last run)


def _build_interp(x_fine: np.ndarray, bfit: float):
    """Band-limited LS upsampling matrix A [W, WC]: rows map coarse samples
    (at xc) of any signal with spectrum in [-bfit, bfit] to the fine grid."""
    key = (x_fine.shape[0], float(x_fine[0]), float(x_fine[-1]), round(bfit))
    if key in _interp_cache:
        return _interp_cache[key]
    x0, x1 = float(x_fine.min()), float(x_fine.max())
    ln = x1 - x0
    xc = np.linspace(x0 - EXT * ln, x1 + EXT * ln, WC)
    f = np.linspace(0.0, bfit, MFREQ)
    s_mat = np.concatenate(
        [np.cos(np.outer(xc - x0, f)), np.sin(np.outer(xc - x0, f))], axis=1
    )
    t_mat = np.concatenate(
        [np.cos(np.outer(x_fine - x0, f)), np.sin(np.outer(x_fine - x0, f))],
        axis=1,
    )
    g = s_mat @ s_mat.T
    a = t_mat @ s_mat.T @ np.linalg.inv(
        g + REG * np.trace(g) / WC * np.eye(WC)
    )
    res = (xc, np.ascontiguousarray(a, dtype=np.float32))
    _interp_cache[key] = res
    return res


def _build_program(r: tuple):
    """Build the per-core Bass program (SPMD; identical on all cores).
    `r` are the 8 Fresnel coefficients, baked in as immediates."""
    if r in _program_cache:
        return _program_cache[r]

    import concourse.tile as tile
    from concourse import bacc, mybir

    f32 = mybir.dt.float32
    bf16 = mybir.dt.bfloat16
    alu = mybir.AluOpType
    act_sin = mybir.ActivationFunctionType.Sin
    act_sq = mybir.ActivationFunctionType.Square

    # Bacc (not raw Bass): its compile() splits multi-sem waits into event
    # semaphores (TRN2 allows 1 wait/instruction), moves matmul waits to
    # ldweights, inserts ACT table loads, and fills extended-ISA bytes.
    nc = bacc.Bacc()
    # theta = k0*g is computed as an exact bf16 triple-split matmul:
    # k0 = kh+km+kl, g = gh+gm+gl (bf16 each); keeping products down to
    # 2^-24 needs rows (kh*gh, kh*gm, km*gh, kh*gl, km*gm, kl*gh) -> K=6
    # per packed half. k0w3 [12, NPART] are the stationary k-patterns,
    # g3 [12, 7*(BPC//PACK)] the moving g rows (host-prepared).
    k0w3 = nc.declare_dram_parameter("k0w3", [12, NPART], bf16, isOutput=False)
    g_3 = nc.declare_dram_parameter(
        "g3", [12, 7 * (BPC // PACK)], bf16, isOutput=False
    )
    # A^T duplicated into both partition halves (rows 0:WC and WC:2*WC),
    # split hi/lo in bf16 ([0]=hi, [1]=lo) for the 3-term bf16 upsample
    amat = nc.declare_dram_parameter("Amat2", [2, NPART, W], bf16, isOutput=False)
    out = nc.declare_dram_parameter("out", [BPC, W], f32, isOutput=True)

    with tile.TileContext(nc) as tc:
        with (
            tc.tile_pool(name="const", bufs=1) as cpool,
            tc.tile_pool(name="theta_sb", bufs=1) as tpool,
            tc.tile_pool(name="state", bufs=2) as spool,
            tc.tile_pool(name="fin", bufs=1) as fpool,
            tc.tile_pool(name="work", bufs=2) as wpool,
            tc.tile_pool(name="thps", bufs=2, space="PSUM") as ppool,
            tc.tile_pool(name="upps", bufs=2, space="PSUM") as upool,
            tc.tile_pool(name="outp", bufs=3) as opool,
        ):
            act_copy = mybir.ActivationFunctionType.Copy
            k0_sb = cpool.tile([12, NPART], bf16)
            nc.sync.dma_start(out=k0_sb, in_=k0w3[:, :])
            g_all = cpool.tile([12, 7 * (BPC // PACK)], bf16)
            nc.sync.dma_start(out=g_all, in_=g_3[:, :])
            a_sb = cpool.tile([NPART, 2, W], bf16)
            nc.sync.dma_start(out=a_sb, in_=amat.rearrange("s p w -> p s w"))

            # ---- phase A: ALL theta matmuls, drained into SBUF via the
            # idle ScalarE so PE retires its recurrence work immediately
            # and upsample matmuls can interleave with the recurrence ----
            thetas = {}
            for chunk in range(NCHUNK):
                nb = NBS[chunk]
                c0 = COFF[chunk]
                for i in range(7, 0, -1):
                    theta = ppool.tile([NPART, nb], f32, tag="theta")
                    gbase = (i - 1) * (BPC // PACK) + c0
                    for t0 in range(0, nb, 512):
                        tw = min(512, nb - t0)
                        nc.tensor.matmul(
                            theta[:, t0 : t0 + tw],
                            k0_sb,
                            g_all[:, gbase + t0 : gbase + t0 + tw],
                            start=True,
                            stop=True,
                        )
                    th_sb = tpool.tile([NPART, nb], f32, tag=f"th{chunk}_{i}")
                    nc.scalar.activation(th_sb, theta, act_copy)
                    thetas[(chunk, i)] = th_sb

            # ---- phase B: the 7-step backward recurrence. The two
            # chunks' chains are independent; interleaving them step-by-step
            # keeps DVE saturated while one chain waits on ACT/GpSimd ----
            st = [dict() for _ in range(NCHUNK)]
            for i in range(7, 0, -1):
                ri = float(r[i - 1])
                for chunk in range(NCHUNK):
                    nb = NBS[chunk]
                    th_sb = thetas[(chunk, i)]
                    psi_s = wpool.tile([NPART, nb], f32, tag=f"psis{chunk}")
                    nc.vector.add_range_wrap(
                        psi_s, th_sb, shift=-TWO_PI, bound=math.pi, period=TWO_PI
                    )
                    psi_c = wpool.tile([NPART, nb], f32, tag=f"psic{chunk}")
                    nc.vector.add_range_wrap(
                        psi_c,
                        th_sb,
                        shift=0.5 * math.pi - TWO_PI,
                        bound=math.pi,
                        period=TWO_PI,
                    )
                    s_t = wpool.tile([NPART, nb], f32, tag=f"s{chunk}")
                    nc.scalar.activation(s_t, psi_s, act_sin)
                    c_t = wpool.tile([NPART, nb], f32, tag=f"c{chunk}")
                    nc.scalar.activation(c_t, psi_c, act_sin)

                    if i == 7:
                        # closed-form first step: u=(r7*c+r8, -r7*s),
                        # v=(c+r7*r8, -s)
                        ur = spool.tile([NPART, nb], f32, tag=f"ur{chunk}")
                        nc.vector.tensor_scalar(
                            ur, c_t, ri, float(r[7]), alu.mult, alu.add
                        )
                        ui = spool.tile([NPART, nb], f32, tag=f"ui{chunk}")
                        nc.vector.tensor_scalar_mul(ui, s_t, -ri)
                        vr = spool.tile([NPART, nb], f32, tag=f"vr{chunk}")
                        nc.vector.tensor_scalar_add(vr, c_t, ri * float(r[7]))
                        vi = spool.tile([NPART, nb], f32, tag=f"vi{chunk}")
                        nc.vector.tensor_scalar_mul(vi, s_t, -1.0)
                        st[chunk] = dict(ur=ur, ui=ui, vr=vr, vi=vi)
                        continue

                    ur, ui, vr, vi = (
                        st[chunk]["ur"], st[chunk]["ui"],
                        st[chunk]["vr"], st[chunk]["vi"],
                    )
                    # T = v * conj(E) : Tr = vr*c + vi*s ; Ti = vi*c - vr*s
                    p1 = wpool.tile([NPART, nb], f32, tag=f"p1{chunk}")
                    nc.vector.tensor_mul(p1, vr, c_t)
                    p2 = wpool.tile([NPART, nb], f32, tag=f"p2{chunk}")
                    nc.vector.tensor_mul(p2, vi, s_t)
                    # two independent muls on the otherwise-idle GpSimd
                    p3 = wpool.tile([NPART, nb], f32, tag=f"p3{chunk}")
                    nc.gpsimd.tensor_mul(p3, vi, c_t)
                    p4 = wpool.tile([NPART, nb], f32, tag=f"p4{chunk}")
                    nc.gpsimd.tensor_mul(p4, vr, s_t)
                    t_r = wpool.tile([NPART, nb], f32, tag=f"Tr{chunk}")
                    nc.vector.tensor_add(t_r, p1, p2)
                    t_i = wpool.tile([NPART, nb], f32, tag=f"Ti{chunk}")
                    nc.vector.tensor_sub(t_i, p3, p4)

                    # u' = r*T + u ; v' = T + r*u   (old u!)
                    ur2 = spool.tile([NPART, nb], f32, tag=f"ur{chunk}")
                    nc.vector.scalar_tensor_tensor(ur2, t_r, ri, ur, alu.mult, alu.add)
                    ui2 = spool.tile([NPART, nb], f32, tag=f"ui{chunk}")
                    nc.vector.scalar_tensor_tensor(ui2, t_i, ri, ui, alu.mult, alu.add)
                    vr2 = spool.tile([NPART, nb], f32, tag=f"vr{chunk}")
                    nc.vector.scalar_tensor_tensor(vr2, ur, ri, t_r, alu.mult, alu.add)
                    vi2 = spool.tile([NPART, nb], f32, tag=f"vi{chunk}")
                    nc.vector.scalar_tensor_tensor(vi2, ui, ri, t_i, alu.mult, alu.add)
                    st[chunk] = dict(ur=ur2, ui=ui2, vr=vr2, vi=vi2)

            plane_splits = []
            for chunk in range(NCHUNK):
                nb = NBS[chunk]
                ur, ui, vr, vi = (
                    st[chunk]["ur"], st[chunk]["ui"],
                    st[chunk]["vr"], st[chunk]["vi"],
                )
                # U2 = ur^2 + ui^2 ; V2 = vr^2 + vi^2 (squares on ScalarE)
                m1 = fpool.tile([NPART, nb], f32, tag="m1")
                nc.scalar.activation(m1, ur, act_sq)
                m2 = fpool.tile([NPART, nb], f32, tag="m2")
                nc.scalar.activation(m2, ui, act_sq)
                u2 = fpool.tile([NPART, nb], f32, tag="u2")
                nc.vector.tensor_add(u2, m1, m2)
                m3 = fpool.tile([NPART, nb], f32, tag="m3")
                nc.scalar.activation(m3, vr, act_sq)
                m4 = fpool.tile([NPART, nb], f32, tag="m4")
                nc.scalar.activation(m4, vi, act_sq)
                v2 = fpool.tile([NPART, nb], f32, tag="v2")
                nc.gpsimd.tensor_add(v2, m3, m4)

                # split planes into bf16 hi + lo (hi-casts on ScalarE)
                u2h = fpool.tile([NPART, nb], bf16, tag=f"u2h{chunk}")
                nc.scalar.activation(u2h, u2, act_copy)
                u2l = fpool.tile([NPART, nb], bf16, tag=f"u2l{chunk}")
                nc.vector.tensor_sub(u2l, u2, u2h)
                v2h = fpool.tile([NPART, nb], bf16, tag=f"v2h{chunk}")
                nc.scalar.activation(v2h, v2, act_copy)
                v2l = fpool.tile([NPART, nb], bf16, tag=f"v2l{chunk}")
                nc.vector.tensor_sub(v2l, v2, v2h)
                plane_splits.append((u2h, u2l, v2h, v2l))

            # ---- phase C: upsample (PE), stage U-plane to SBUF (ScalarE),
            # reciprocal (DVE), multiply (GpSimd), store (DMA) ----
            for chunk in range(NCHUNK):
                nb = NBS[chunk]
                u2h, u2l, v2h, v2l = plane_splits[chunk]
                for half in range(PACK):
                    for bsub in range(nb // 128):
                        row0 = half * (BPC // 2) + COFF[chunk] + bsub * 128
                        bs = slice(bsub * 128, (bsub + 1) * 128)
                        ps = slice(half * WC, (half + 1) * WC)
                        for w0, ws in ((0, 512), (512, W - 512)):
                            a_h = a_sb[ps, 0, w0 : w0 + ws]
                            a_l = a_sb[ps, 1, w0 : w0 + ws]
                            up_u = upool.tile([128, ws], f32, tag="upu")
                            nc.tensor.matmul(
                                up_u, u2h[ps, bs], a_h, start=True, stop=False
                            )
                            nc.tensor.matmul(
                                up_u, u2h[ps, bs], a_l, start=False, stop=False
                            )
                            nc.tensor.matmul(
                                up_u, u2l[ps, bs], a_h, start=False, stop=True
                            )
                            su = opool.tile([128, ws], f32, tag="su")
                            nc.scalar.activation(su, up_u, act_copy)
                            up_v = upool.tile([128, ws], f32, tag="upv")
                            nc.tensor.matmul(
                                up_v, v2h[ps, bs], a_h, start=True, stop=False
                            )
                            nc.tensor.matmul(
                                up_v, v2h[ps, bs], a_l, start=False, stop=False
                            )
                            nc.tensor.matmul(
                                up_v, v2l[ps, bs], a_h, start=False, stop=True
                            )
                            inv = opool.tile([128, ws], f32, tag="inv")
                            nc.vector.reciprocal_approx_fast(out=inv, in_=up_v)
                            ro = opool.tile([128, ws], f32, tag="ro")
                            nc.gpsimd.tensor_mul(ro, su, inv)
                            nc.sync.dma_start(
                                out=out[row0 : row0 + 128, w0 : w0 + ws], in_=ro
                            )

    # the axon/bass2jax run path asserts finalized; Bacc.finalize() also runs
    # compile() (event-sem wait splitting, reg alloc, ACT table loads, ISA bytes)
    nc.finalize()

    _program_cache[r] = nc
    return nc


def _scalar_index(arr: np.ndarray, name: str) -> float:
    """Wavelength-constant real refractive index from a [W] complex array."""
    a = np.asarray(arr)
    v = complex(a.flat[0])
    if not np.allclose(a, v, rtol=1e-6, atol=1e-6):
        raise ValueError(f"{name}: kernel requires wavelength-constant index")
    if abs(v.imag) > 1e-6 * max(1.0, abs(v.real)):
        raise ValueError(f"{name}: kernel requires a real refractive index")
    return float(v.real)


def kernel(d_phys, n_Si, n_SiO2, n_Si3N4, lam):
    from concourse.bass_utils import run_bass_kernel_spmd

    d = np.ascontiguousarray(np.asarray(d_phys), dtype=np.float32)
    assert d.shape == (B, 7), d.shape
    lam_np = np.asarray(lam, dtype=np.float64)
    assert lam_np.shape == (W,)

    nsi = _scalar_index(n_Si, "n_Si")
    nox = _scalar_index(n_SiO2, "n_SiO2")
    nni = _scalar_index(n_Si3N4, "n_Si3N4")
    layers = [1.0, nox, nni, nox, nni, nox, nni, nox, nsi]
    r = tuple(
        (layers[i - 1] - layers[i]) / (layers[i - 1] + layers[i])
        for i in range(1, 9)
    )

    # g = 2*n_i*d_i per layer, [B, 7]
    nvec = np.array(layers[1:8], dtype=np.float64)
    g = (2.0 * d.astype(np.float64) * nvec[None, :]).astype(np.float32)

    x_fine = 1.0 / lam_np
    bfit_actual = 2.0 * TWO_PI * float(g.astype(np.float64).sum(axis=1).max())
    bfit = max(B_DESIGN, bfit_actual * 1.02)
    xc, a_fine = _build_interp(x_fine, bfit)  # a_fine: [W, WC]

    # sanity: wrapped phase args must stay within +-3*pi for the one-period
    # range wrap (theta in [0, theta_max])
    theta_max = TWO_PI * float(xc.max()) * float(g.max())
    assert theta_max <= 5.0 * math.pi - 0.2, theta_max

    import ml_dtypes

    bf = ml_dtypes.bfloat16

    def split3(x):
        h = x.astype(bf)
        m = (x - h.astype(np.float64)).astype(bf)
        l = (x - h.astype(np.float64) - m.astype(np.float64)).astype(bf)
        return h, m, l

    k0c = TWO_PI * xc  # float64
    kh, km, kl = split3(k0c)
    k_rows = (kh, kh, km, kh, km, kl)
    g_rows_parts = (0, 1, 0, 2, 1, 0)  # indices into (gh, gm, gl)
    k0w3 = np.zeros((12, NPART), dtype=bf)
    for rr in range(6):
        k0w3[rr, :WC] = k_rows[rr]
        k0w3[6 + rr, WC:] = k_rows[rr]

    a2 = np.concatenate([a_fine.T, a_fine.T], axis=0)  # [2*WC, W] fp32
    a_hi = a2.astype(bf)
    a_lo = (a2 - a_hi.astype(np.float32)).astype(bf)
    amat2 = np.ascontiguousarray(np.stack([a_hi, a_lo], axis=0))

    seg = BPC // PACK
    g_all = g.astype(np.float64).reshape(NCORES, PACK, seg, 7)
    in_maps = []
    for c in range(NCORES):
        g3 = np.zeros((12, 7 * seg), dtype=bf)
        for h in range(PACK):
            gh, gm, gl = split3(g_all[c, h])  # [seg, 7] each
            gparts = (gh, gm, gl)
            for rr in range(6):
                row = h * 6 + rr
                src_part = gparts[g_rows_parts[rr]]
                for i in range(7):
                    g3[row, i * seg : (i + 1) * seg] = src_part[:, i]
        in_maps.append({"g3": g3, "k0w3": k0w3, "Amat2": amat2})

    nc = _build_program(r)
    _last_launch["nc"] = nc
    _last_launch["in_maps"] = in_maps
    res = run_bass_kernel_spmd(nc, in_maps, list(range(NCORES)))
    outs = [res.results[c]["out"] for c in range(NCORES)]
    return np.concatenate(outs, axis=0)


# revision 47
# speedup vs baseline: 1.0510x; 1.0510x over previous
"""Trainium2 Bass kernel for transfer-matrix reflectometry (DifferentiableKLA).

Math: for each batch row b (7 film thicknesses) and wavelength w, the
reference computes R = |M10/M00|^2 of an 8-interface transfer-matrix
product. Dividing the backward (Rouard) recurrence through by v and
rescaling by conj(E_i) each step (the overall complex scale cancels in the
ratio) gives the division-free form over x = 1/lambda:

    u, v   <- r_8, 1
    for i = 7..1:   T = v * conj(E_i);  u' = r_i*T + u;  v' = T + r_i*u
    R = |u|^2 / |v|^2,   E_i = exp(i * theta_i), theta_i = 2*pi*x*g_i,
    g_i = 2*n_i*d_i,   r_i = (n_{i-1}-n_i)/(n_{i-1}+n_i)  (real scalars
    since the refractive indices are wavelength-constant and real).

Key accel trick: U2=|u|^2 and V2=|v|^2 are *exactly band-limited* in x
(finite exponential sums, |freq| <= 4*pi*sum_j g_j), so they are computed
on a coarse WC-point grid in x and upsampled to the 801-point grid with a
precomputed band-limited least-squares matrix — evaluated as a TensorE
matmul that simultaneously transposes to the [batch, wavelength] output
layout. Only the final reciprocal+multiply runs on the fine grid.

Layout: coarse grid on partitions, batch along the free dim, two
independent batch halves packed as 2 x WC=64 partition groups (rank-2
matmul builds theta for both halves at once).
"""

import math

import numpy as np

W = 801
B = 16384
NCORES = 8
BPC = B // NCORES  # 2048 batch rows per core
WC = 64  # coarse wavelength nodes
PACK = 2  # batch halves packed along partitions
NPART = WC * PACK  # 128
NBS = (512, 512)  # chunk widths (step-interleaved in the recurrence)
COFF = (0, 512)  # column offsets per chunk
NCHUNK = len(NBS)
EXT = 0.05  # coarse-grid extension fraction beyond the x band
REG = 1e-10  # LS ridge regularization
MFREQ = 6000  # frequency samples for the LS fit
TWO_PI = 2.0 * math.pi
# worst-case band limit for d < 200nm stacks (4*pi*sum(2*n_j*200)), padded
B_DESIGN = 2.0 * TWO_PI * (2.0 * (1.46 * 4 + 2.0 * 3) * 200.0) * 1.02

_interp_cache: dict = {}
_program_cache: dict = {}
_last_launch: dict = {}  # debug/profiling hook (nc, in_maps of last run)


def _build_interp(x_fine: np.ndarray, bfit: float):
    """Band-limited LS upsampling matrix A [W, WC]: rows map coarse samples
    (at xc) of any signal with spectrum in [-bfit, bfit] to the fine grid."""
    key = (x_fine.shape[0], float(x_fine[0]), float(x_fine[-1]), round(bfit))
    if key in _interp_cache:
        return _interp_cache[key]
    x0, x1 = float(x_fine.min()), float(x_fine.max())
    ln = x1 - x0
    xc = np.linspace(x0 - EXT * ln, x1 + EXT * ln, WC)
    f = np.linspace(0.0, bfit, MFREQ)
    s_mat = np.concatenate(
        [np.cos(np.outer(xc - x0, f)), np.sin(np.outer(xc - x0, f))], axis=1
    )
    t_mat = np.concatenate(
        [np.cos(np.outer(x_fine - x0, f)), np.sin(np.outer(x_fine - x0, f))],
        axis=1,
    )
    g = s_mat @ s_mat.T
    a = t_mat @ s_mat.T @ np.linalg.inv(
        g + REG * np.trace(g) / WC * np.eye(WC)
    )
    res = (xc, np.ascontiguousarray(a, dtype=np.float32))
    _interp_cache[key] = res
    return res


def _build_program(r: tuple):
    """Build the per-core Bass program (SPMD; identical on all cores).
    `r` are the 8 Fresnel coefficients, baked in as immediates."""
    if r in _program_cache:
        return _program_cache[r]

    import concourse.tile as tile
    from concourse import bacc, mybir

    f32 = mybir.dt.float32
    bf16 = mybir.dt.bfloat16
    alu = mybir.AluOpType
    act_sin = mybir.ActivationFunctionType.Sin
    act_sq = mybir.ActivationFunctionType.Square

    # Bacc (not raw Bass): its compile() splits multi-sem waits into event
    # semaphores (TRN2 allows 1 wait/instruction), moves matmul waits to
    # ldweights, inserts ACT table loads, and fills extended-ISA bytes.
    nc = bacc.Bacc()
    # theta = k0*g is computed as an exact bf16 triple-split matmul:
    # k0 = kh+km+kl, g = gh+gm+gl (bf16 each); keeping products down to
    # 2^-24 needs rows (kh*gh, kh*gm, km*gh, kh*gl, km*gm, kl*gh) -> K=6
    # per packed half. k0w3 [12, NPART] are the stationary k-patterns,
    # g3 [12, 7*(BPC//PACK)] the moving g rows (host-prepared).
    k0w3 = nc.declare_dram_parameter("k0w3", [12, NPART], bf16, isOutput=False)
    g_3 = nc.declare_dram_parameter(
        "g3", [12, 7 * (BPC // PACK)], bf16, isOutput=False
    )
    # A^T duplicated into both partition halves (rows 0:WC and WC:2*WC),
    # split hi/lo in bf16 ([0]=hi, [1]=lo) for the 3-term bf16 upsample
    amat = nc.declare_dram_parameter("Amat2", [2, NPART, W], bf16, isOutput=False)
    out = nc.declare_dram_parameter("out", [BPC, W], f32, isOutput=True)

    with tile.TileContext(nc) as tc:
        with (
            tc.tile_pool(name="const", bufs=1) as cpool,
            tc.tile_pool(name="theta_sb", bufs=1) as tpool,
            tc.tile_pool(name="state", bufs=2) as spool,
            tc.tile_pool(name="fin", bufs=1) as fpool,
            tc.tile_pool(name="work", bufs=2) as wpool,
            tc.tile_pool(name="thps", bufs=2, space="PSUM") as ppool,
            tc.tile_pool(name="upps", bufs=2, space="PSUM") as upool,
            tc.tile_pool(name="outp", bufs=3) as opool,
        ):
            act_copy = mybir.ActivationFunctionType.Copy
            k0_sb = cpool.tile([12, NPART], bf16)
            nc.sync.dma_start(out=k0_sb, in_=k0w3[:, :])
            g_all = cpool.tile([12, 7 * (BPC // PACK)], bf16)
            nc.sync.dma_start(out=g_all, in_=g_3[:, :])
            a_sb = cpool.tile([NPART, 2, W], bf16)
            nc.sync.dma_start(out=a_sb, in_=amat.rearrange("s p w -> p s w"))

            # ---- phase A: ALL theta matmuls, drained into SBUF via the
            # idle ScalarE so PE retires its recurrence work immediately
            # and upsample matmuls can interleave with the recurrence ----
            thetas = {}
            for chunk in range(NCHUNK):
                nb = NBS[chunk]
                c0 = COFF[chunk]
                for i in range(7, 0, -1):
                    theta = ppool.tile([NPART, nb], f32, tag="theta")
                    gbase = (i - 1) * (BPC // PACK) + c0
                    for t0 in range(0, nb, 512):
                        tw = min(512, nb - t0)
                        nc.tensor.matmul(
                            theta[:, t0 : t0 + tw],
                            k0_sb,
                            g_all[:, gbase + t0 : gbase + t0 + tw],
                            start=True,
                            stop=True,
                        )
                    th_sb = tpool.tile([NPART, nb], f32, tag=f"th{chunk}_{i}")
                    nc.scalar.activation(th_sb, theta, act_copy)
                    thetas[(chunk, i)] = th_sb

            # ---- phase B: the 7-step backward recurrence. The two
            # chunks' chains are independent; interleaving them step-by-step
            # keeps DVE saturated while one chain waits on ACT/GpSimd ----
            st = [dict() for _ in range(NCHUNK)]
            for i in range(7, 0, -1):
                ri = float(r[i - 1])
                for chunk in range(NCHUNK):
                    nb = NBS[chunk]
                    th_sb = thetas[(chunk, i)]
                    psi_s = wpool.tile([NPART, nb], f32, tag=f"psis{chunk}")
                    nc.vector.add_range_wrap(
                        psi_s, th_sb, shift=-TWO_PI, bound=math.pi, period=TWO_PI
                    )
                    psi_c = wpool.tile([NPART, nb], f32, tag=f"psic{chunk}")
                    nc.vector.add_range_wrap(
                        psi_c,
                        th_sb,
                        shift=0.5 * math.pi - TWO_PI,
                        bound=math.pi,
                        period=TWO_PI,
                    )
                    s_t = wpool.tile([NPART, nb], f32, tag=f"s{chunk}")
                    nc.scalar.activation(s_t, psi_s, act_sin)
                    c_t = wpool.tile([NPART, nb], f32, tag=f"c{chunk}")
                    nc.scalar.activation(c_t, psi_c, act_sin)

                    if i == 7:
                        # closed-form first step: u=(r7*c+r8, -r7*s),
                        # v=(c+r7*r8, -s)
                        ur = spool.tile([NPART, nb], f32, tag=f"ur{chunk}")
                        nc.vector.tensor_scalar(
                            ur, c_t, ri, float(r[7]), alu.mult, alu.add
                        )
                        ui = spool.tile([NPART, nb], f32, tag=f"ui{chunk}")
                        nc.vector.tensor_scalar_mul(ui, s_t, -ri)
                        vr = spool.tile([NPART, nb], f32, tag=f"vr{chunk}")
                        nc.vector.tensor_scalar_add(vr, c_t, ri * float(r[7]))
                        vi = spool.tile([NPART, nb], f32, tag=f"vi{chunk}")
                        nc.vector.tensor_scalar_mul(vi, s_t, -1.0)
                        st[chunk] = dict(ur=ur, ui=ui, vr=vr, vi=vi)
                        continue

                    ur, ui, vr, vi = (
                        st[chunk]["ur"], st[chunk]["ui"],
                        st[chunk]["vr"], st[chunk]["vi"],
                    )
                    # T = v * conj(E) : Tr = vr*c + vi*s ; Ti = vi*c - vr*s
                    p1 = wpool.tile([NPART, nb], f32, tag=f"p1{chunk}")
                    nc.vector.tensor_mul(p1, vr, c_t)
                    p2 = wpool.tile([NPART, nb], f32, tag=f"p2{chunk}")
                    nc.vector.tensor_mul(p2, vi, s_t)
                    p3 = wpool.tile([NPART, nb], f32, tag=f"p3{chunk}")
                    nc.vector.tensor_mul(p3, vi, c_t)
                    p4 = wpool.tile([NPART, nb], f32, tag=f"p4{chunk}")
                    nc.vector.tensor_mul(p4, vr, s_t)
                    t_r = wpool.tile([NPART, nb], f32, tag=f"Tr{chunk}")
                    nc.vector.tensor_add(t_r, p1, p2)
                    t_i = wpool.tile([NPART, nb], f32, tag=f"Ti{chunk}")
                    nc.vector.tensor_sub(t_i, p3, p4)

                    # u' = r*T + u ; v' = T + r*u   (old u!)
                    ur2 = spool.tile([NPART, nb], f32, tag=f"ur{chunk}")
                    nc.vector.scalar_tensor_tensor(ur2, t_r, ri, ur, alu.mult, alu.add)
                    ui2 = spool.tile([NPART, nb], f32, tag=f"ui{chunk}")
                    nc.vector.scalar_tensor_tensor(ui2, t_i, ri, ui, alu.mult, alu.add)
                    vr2 = spool.tile([NPART, nb], f32, tag=f"vr{chunk}")
                    nc.vector.scalar_tensor_tensor(vr2, ur, ri, t_r, alu.mult, alu.add)
                    vi2 = spool.tile([NPART, nb], f32, tag=f"vi{chunk}")
                    nc.vector.scalar_tensor_tensor(vi2, ui, ri, t_i, alu.mult, alu.add)
                    st[chunk] = dict(ur=ur2, ui=ui2, vr=vr2, vi=vi2)

            plane_splits = []
            for chunk in range(NCHUNK):
                nb = NBS[chunk]
                ur, ui, vr, vi = (
                    st[chunk]["ur"], st[chunk]["ui"],
                    st[chunk]["vr"], st[chunk]["vi"],
                )
                # U2 = ur^2 + ui^2 ; V2 = vr^2 + vi^2 (squares on ScalarE)
                m1 = fpool.tile([NPART, nb], f32, tag="m1")
                nc.scalar.activation(m1, ur, act_sq)
                m2 = fpool.tile([NPART, nb], f32, tag="m2")
                nc.scalar.activation(m2, ui, act_sq)
                u2 = fpool.tile([NPART, nb], f32, tag="u2")
                nc.vector.tensor_add(u2, m1, m2)
                m3 = fpool.tile([NPART, nb], f32, tag="m3")
                nc.scalar.activation(m3, vr, act_sq)
                m4 = fpool.tile([NPART, nb], f32, tag="m4")
                nc.scalar.activation(m4, vi, act_sq)
                v2 = fpool.tile([NPART, nb], f32, tag="v2")
                nc.vector.tensor_add(v2, m3, m4)

                # split planes into bf16 hi + lo (hi-casts on ScalarE)
                u2h = fpool.tile([NPART, nb], bf16, tag=f"u2h{chunk}")
                nc.scalar.activation(u2h, u2, act_copy)
                u2l = fpool.tile([NPART, nb], bf16, tag=f"u2l{chunk}")
                nc.vector.tensor_sub(u2l, u2, u2h)
                v2h = fpool.tile([NPART, nb], bf16, tag=f"v2h{chunk}")
                nc.scalar.activation(v2h, v2, act_copy)
                v2l = fpool.tile([NPART, nb], bf16, tag=f"v2l{chunk}")
                nc.vector.tensor_sub(v2l, v2, v2h)
                plane_splits.append((u2h, u2l, v2h, v2l))

            # ---- phase C: upsample (PE), stage U-plane to SBUF (ScalarE),
            # reciprocal (DVE), multiply (GpSimd), store (DMA) ----
            for chunk in range(NCHUNK):
                nb = NBS[chunk]
                u2h, u2l, v2h, v2l = plane_splits[chunk]
                for half in range(PACK):
                    for bsub in range(nb // 128):
                        row0 = half * (BPC // 2) + COFF[chunk] + bsub * 128
                        bs = slice(bsub * 128, (bsub + 1) * 128)
                        ps = slice(half * WC, (half + 1) * WC)
                        for w0, ws in ((0, 512), (512, W - 512)):
                            a_h = a_sb[ps, 0, w0 : w0 + ws]
                            a_l = a_sb[ps, 1, w0 : w0 + ws]
                            up_u = upool.tile([128, ws], f32, tag="upu")
                            nc.tensor.matmul(
                                up_u, u2h[ps, bs], a_h, start=True, stop=False
                            )
                            nc.tensor.matmul(
                                up_u, u2h[ps, bs], a_l, start=False, stop=False
                            )
                            nc.tensor.matmul(
                                up_u, u2l[ps, bs], a_h, start=False, stop=True
                            )
                            su = opool.tile([128, ws], f32, tag="su")
                            nc.scalar.activation(su, up_u, act_copy)
                            up_v = upool.tile([128, ws], f32, tag="upv")
                            nc.tensor.matmul(
                                up_v, v2h[ps, bs], a_h, start=True, stop=False
                            )
                            nc.tensor.matmul(
                                up_v, v2h[ps, bs], a_l, start=False, stop=False
                            )
                            nc.tensor.matmul(
                                up_v, v2l[ps, bs], a_h, start=False, stop=True
                            )
                            inv = opool.tile([128, ws], f32, tag="inv")
                            nc.vector.reciprocal_approx_fast(out=inv, in_=up_v)
                            ro = opool.tile([128, ws], f32, tag="ro")
                            nc.vector.tensor_mul(ro, su, inv)
                            nc.sync.dma_start(
                                out=out[row0 : row0 + 128, w0 : w0 + ws], in_=ro
                            )

    # the axon/bass2jax run path asserts finalized; Bacc.finalize() also runs
    # compile() (event-sem wait splitting, reg alloc, ACT table loads, ISA bytes)
    nc.finalize()

    _program_cache[r] = nc
    return nc


def _scalar_index(arr: np.ndarray, name: str) -> float:
    """Wavelength-constant real refractive index from a [W] complex array."""
    a = np.asarray(arr)
    v = complex(a.flat[0])
    if not np.allclose(a, v, rtol=1e-6, atol=1e-6):
        raise ValueError(f"{name}: kernel requires wavelength-constant index")
    if abs(v.imag) > 1e-6 * max(1.0, abs(v.real)):
        raise ValueError(f"{name}: kernel requires a real refractive index")
    return float(v.real)


def kernel(d_phys, n_Si, n_SiO2, n_Si3N4, lam):
    from concourse.bass_utils import run_bass_kernel_spmd

    d = np.ascontiguousarray(np.asarray(d_phys), dtype=np.float32)
    assert d.shape == (B, 7), d.shape
    lam_np = np.asarray(lam, dtype=np.float64)
    assert lam_np.shape == (W,)

    nsi = _scalar_index(n_Si, "n_Si")
    nox = _scalar_index(n_SiO2, "n_SiO2")
    nni = _scalar_index(n_Si3N4, "n_Si3N4")
    layers = [1.0, nox, nni, nox, nni, nox, nni, nox, nsi]
    r = tuple(
        (layers[i - 1] - layers[i]) / (layers[i - 1] + layers[i])
        for i in range(1, 9)
    )

    # g = 2*n_i*d_i per layer, [B, 7]
    nvec = np.array(layers[1:8], dtype=np.float64)
    g = (2.0 * d.astype(np.float64) * nvec[None, :]).astype(np.float32)

    x_fine = 1.0 / lam_np
    bfit_actual = 2.0 * TWO_PI * float(g.astype(np.float64).sum(axis=1).max())
    bfit = max(B_DESIGN, bfit_actual * 1.02)
    xc, a_fine = _build_interp(x_fine, bfit)  # a_fine: [W, WC]

    # sanity: wrapped phase args must stay within +-3*pi for the one-period
    # range wrap (theta in [0, theta_max])
    theta_max = TWO_PI * float(xc.max()) * float(g.max())
    assert theta_max <= 5.0 * math.pi - 0.2, theta_max

    import ml_dtypes

    bf = ml_dtypes.bfloat16

    def split3(x):
        h = x.astype(bf)
        m = (x - h.astype(np.float64)).astype(bf)
        l = (x - h.astype(np.float64) - m.astype(np.float64)).astype(bf)
        return h, m, l

    k0c = TWO_PI * xc  # float64
    kh, km, kl = split3(k0c)
    k_rows = (kh, kh, km, kh, km, kl)
    g_rows_parts = (0, 1, 0, 2, 1, 0)  # indices into (gh, gm, gl)
    k0w3 = np.zeros((12, NPART), dtype=bf)
    for rr in range(6):
        k0w3[rr, :WC] = k_rows[rr]
        k0w3[6 + rr, WC:] = k_rows[rr]

    a2 = np.concatenate([a_fine.T, a_fine.T], axis=0)  # [2*WC, W] fp32
    a_hi = a2.astype(bf)
    a_lo = (a2 - a_hi.astype(np.float32)).astype(bf)
    amat2 = np.ascontiguousarray(np.stack([a_hi, a_lo], axis=0))

    seg = BPC // PACK
    g_all = g.astype(np.float64).reshape(NCORES, PACK, seg, 7)
    in_maps = []
    for c in range(NCORES):
        g3 = np.zeros((12, 7 * seg), dtype=bf)
        for h in range(PACK):
            gh, gm, gl = split3(g_all[c, h])  # [seg, 7] each
            gparts = (gh, gm, gl)
            for rr in range(6):
                row = h * 6 + rr
                src_part = gparts[g_rows_parts[rr]]
                for i in range(7):
                    g3[row, i * seg : (i + 1) * seg] = src_part[:, i]
        in_maps.append({"g3": g3, "k0w3": k0w3, "Amat2": amat2})

    nc = _build_program(r)
    _last_launch["nc"] = nc
    _last_launch["in_maps"] = in_maps
    res = run_bass_kernel_spmd(nc, in_maps, list(range(NCORES)))
    outs = [res.results[c]["out"] for c in range(NCORES)]
    return np.concatenate(outs, axis=0)
